# revision 9
# baseline (speedup 1.0000x reference)
"""Trainium2 Bass kernel: causal multi-head self-attention (B=4, T=4096, D=128, H=4, dh=32).

Sharding: 8 cores = 4 batches x 2 head-pairs. Core c handles batch c//2, heads
{2*(c%2), 2*(c%2)+1}. Each core returns a partial output projection (its 2 heads'
contribution); the host sums the two partials per batch.

Algorithm per (head, q-super of 512 queries):
  scores^T[j, q] = K^T-block (zero-padded K=128 lhsT) @ Q^T  -> PSUM
  E = exp(scores^T * 1/sqrt(dh))  (fp32r out, no max-subtraction: scores are O(3))
  E *= causal mask on diagonal blocks
  O^T[d|l, q] += [V | ones | 0]^T-block @ E   (accumulated over j-blocks in PSUM;
                                               row 32 = softmax denominator l)
  Y_h[q, o] = (O^T slice)^T @ W_out-slice ; out = Y_h0/l0 + Y_h1/l1 (per-partition scale)
"""

import math
import numpy as np

import concourse.bass as bass
import concourse.bacc as bacc
import concourse.mybir as mybir
import concourse.tile as tile
from concourse import bass_utils

F32 = mybir.dt.float32
F32R = mybir.dt.float32r
Exp = mybir.ActivationFunctionType.Exp

B, T, D = 4, 4096, 128
H, DH = 4, 32
NCORES = 8
NQS = T // 512          # 8 q-supers
NJB = T // 128          # 32 j-blocks
GROUP = 3               # j-blocks per S-psum group (3 banks per slot, double buffered)
SCALE = 1.0 / math.sqrt(DH)

# matmul dtypes (switchable): S-stage (Q/K) and O-stage (V/E)
DT_S = F32R
DT_O = F32R


def _round_f32r(x: np.ndarray) -> np.ndarray:
    """Round-to-nearest-even fp32 -> fp32r (11 explicit mantissa bits, low 12 dropped)."""
    u = np.ascontiguousarray(x, dtype=np.float32).view(np.uint32)
    half = np.uint32(1 << 11)
    lsb = (u >> np.uint32(12)) & np.uint32(1)
    u = ((u + half - np.uint32(1) + lsb) >> np.uint32(12)) << np.uint32(12)
    return u.view(np.float32)


def _to_dt(x: np.ndarray, dt) -> np.ndarray:
    if dt == F32R:
        return _round_f32r(x)
    return np.ascontiguousarray(x, dtype=np.float32)


def build_program() -> bacc.Bacc:
    nc = bacc.Bacc("TRN2", target_bir_lowering=False, debug=False, num_devices=NCORES)

    # ---- DRAM I/O (per core) ----
    xt_d = nc.dram_tensor("xt", [D, T], DT_S, kind="ExternalInput").ap()
    wq_d = [nc.dram_tensor(f"wq{h}", [D, 128], DT_S, kind="ExternalInput").ap() for h in range(2)]
    wk_d = [nc.dram_tensor(f"wk{h}", [D, 128], DT_S, kind="ExternalInput").ap() for h in range(2)]
    wv_d = nc.dram_tensor("wv", [D, 64], DT_S, kind="ExternalInput").ap()
    wo_d = [nc.dram_tensor(f"wo{h}", [128, 128], DT_S, kind="ExternalInput").ap() for h in range(2)]
    mask_d = nc.dram_tensor("mask", [128, 2048], DT_O, kind="ExternalInput").ap()
    vinit_d = nc.dram_tensor("vinit", [128, T], DT_O, kind="ExternalInput").ap()
    y_d = nc.dram_tensor("y", [T, D], F32, kind="ExternalOutput").ap()
    l_d = nc.dram_tensor("ldram", [2, NQS, 512], F32, kind="Internal").ap()

    with tile.TileContext(nc) as tc:
        with (
            tc.tile_pool(name="const", bufs=1) as cpool,
            tc.tile_pool(name="epool", bufs=2) as epool,
            tc.tile_pool(name="work", bufs=3) as wpool,
            tc.tile_pool(name="psS", bufs=2, space="PSUM") as psS,
            tc.tile_pool(name="psO", bufs=1, space="PSUM") as psO,
            tc.tile_pool(name="psB", bufs=1, space="PSUM") as psB,
        ):
            # ---- persistent SBUF ----
            xt = cpool.tile([D, T], DT_S)
            wq = [cpool.tile([D, 128], DT_S, name=f"wq{h}", tag=f"wq{h}") for h in range(2)]
            wk = [cpool.tile([D, 128], DT_S, name=f"wk{h}", tag=f"wk{h}") for h in range(2)]
            wv = cpool.tile([D, 64], DT_S)
            wo = [cpool.tile([128, 128], DT_S, name=f"wo{h}", tag=f"wo{h}") for h in range(2)]
            mask = cpool.tile([128, 2048], DT_O)
            qt = [cpool.tile([128, T], DT_S, name=f"qt{h}", tag=f"qt{h}") for h in range(2)]
            ktz = [cpool.tile([128, T], DT_S, name=f"ktz{h}", tag=f"ktz{h}") for h in range(2)]
            vx = [cpool.tile([128, T], DT_O, name=f"vx{h}", tag=f"vx{h}") for h in range(2)]
            osb = [cpool.tile([128, T], DT_S, name=f"osb{h}", tag=f"osb{h}") for h in range(2)]
            lcol = [cpool.tile([128, NJB], F32, name=f"lcol{h}", tag=f"lcol{h}") for h in range(2)]
            rl = [cpool.tile([128, NJB], F32, name=f"rl{h}", tag=f"rl{h}") for h in range(2)]
            ytmp = cpool.tile([128, T], F32)

            # ---- weight / mask loads ----
            for h in range(2):
                nc.sync.dma_start(wq[h][:, :], wq_d[h][:, :])
                nc.sync.dma_start(wk[h][:, :], wk_d[h][:, :])
                nc.sync.dma_start(wo[h][:, :], wo_d[h][:, :])
            nc.sync.dma_start(wv[:, :], wv_d[:, :])
            nc.sync.dma_start(mask[:, :], mask_d[:, :])
            # vx pre-pattern from host: [0]*32 | 1.0 | [0]*95 per j-block; V values DMAed on top
            nc.sync.dma_start(vx[0][:, :], vinit_d[:, :])
            nc.sync.dma_start(vx[1][:, :], vinit_d[:, :])

            def emit_qkv(qs):
                sl = slice(512 * qs, 512 * (qs + 1))
                nc.sync.dma_start(xt[:, sl], xt_d[:, sl])
                for dst, w in ((qt[0], wq[0]), (qt[1], wq[1]), (ktz[0], wk[0]), (ktz[1], wk[1])):
                    p = psB.tile([128, 512], F32, name="p", tag="psb")
                    nc.tensor.matmul(p[:, :], w[:, :], xt[:, sl], start=True, stop=True)
                    nc.vector.tensor_copy(dst[:, sl], p[:, :])
                for jb in range(4 * qs, 4 * qs + 4):
                    jsl = slice(128 * jb, 128 * (jb + 1))
                    p = psB.tile([128, 512], F32, name="p", tag="psb")
                    nc.tensor.matmul(p[:, 0:64], xt[:, jsl], wv[:, :], start=True, stop=True)
                    nc.vector.tensor_copy(vx[0][:, 128 * jb : 128 * jb + 32], p[:, 0:32])
                    nc.vector.tensor_copy(vx[1][:, 128 * jb : 128 * jb + 32], p[:, 32:64])

            def emit_attn(h, qs):
                qsl = slice(512 * qs, 512 * (qs + 1))
                njb = 4 * (qs + 1)
                o_ps = psO.tile([128, 512], F32, name="o_ps", tag="po")
                jb0 = 0
                while jb0 < njb:
                    n = min(GROUP, njb - jb0)
                    s_ps = psS.tile([128, 512 * GROUP], F32, name="s_ps", tag="s")
                    for k in range(n):
                        jb = jb0 + k
                        nc.tensor.matmul(
                            s_ps[:, 512 * k : 512 * (k + 1)],
                            ktz[h][:, 128 * jb : 128 * (jb + 1)],
                            qt[h][:, qsl],
                            start=True, stop=True,
                        )
                    e = epool.tile([128, 512 * GROUP], DT_O, name="e", tag="e")
                    nc.scalar.activation(e[:, 0 : 512 * n], s_ps[:, 0 : 512 * n], Exp, scale=SCALE)
                    for k in range(n):
                        jb = jb0 + k
                        g = jb - 4 * qs
                        if g >= 0:
                            esl = slice(512 * k, 512 * (k + 1))
                            nc.vector.tensor_mul(e[:, esl], e[:, esl], mask[:, 512 * g : 512 * (g + 1)])
                    for k in range(n):
                        jb = jb0 + k
                        nc.tensor.matmul(
                            o_ps[:, :],
                            vx[h][:, 128 * jb : 128 * (jb + 1)],
                            e[:, 512 * k : 512 * (k + 1)],
                            start=(jb == 0), stop=(jb == njb - 1),
                        )
                    jb0 += n
                # evacuate: full 128 rows (rows 33..127 are zeros); l row separately in f32
                nc.vector.tensor_copy(osb[h][:, qsl], o_ps[:, :])
                lrow = wpool.tile([1, 512], F32, name="lrow", tag="lrow")
                nc.vector.tensor_copy(lrow[0:1, :], o_ps[32:33, :])
                # l -> per-partition column layout via DRAM bounce + reciprocal
                d1 = nc.sync.dma_start(l_d[h, qs, :], lrow[0:1, :])
                lsrc = l_d[h, qs, :].rearrange("(b c) -> c b", c=128)
                d2 = nc.sync.dma_start(lcol[h][:, 4 * qs : 4 * qs + 4], lsrc)
                tile.add_dep_helper(d2.ins, d1.ins, reason="l dram bounce RAW")
                nc.vector.reciprocal(rl[h][:, 4 * qs : 4 * qs + 4], lcol[h][:, 4 * qs : 4 * qs + 4])

            def emit_proj(h, qs):
                for qb in range(4 * qs, 4 * qs + 4):
                    bsl = slice(128 * qb, 128 * (qb + 1))
                    p = psB.tile([128, 512], F32, name="p", tag="psb")
                    nc.tensor.matmul(p[:, 0:128], osb[h][:, bsl], wo[h][:, :], start=True, stop=True)
                    if h == 0:
                        nc.vector.tensor_scalar_mul(ytmp[:, bsl], p[:, 0:128], rl[0][:, qb : qb + 1])
                    else:
                        ty = wpool.tile([128, 128], F32, name="ty", tag="ty")
                        nc.vector.scalar_tensor_tensor(
                            ty[:, :], p[:, 0:128], rl[1][:, qb : qb + 1], ytmp[:, bsl],
                            op0=mybir.AluOpType.mult, op1=mybir.AluOpType.add,
                        )
                        nc.sync.dma_start(y_d[bsl, :], ty[:, :])

            # ---- phase 1: qkv (interleaved) + head-0 attention + head-0 projections ----
            with nc.named_scope("h0"):
                for qs in range(NQS):
                    emit_qkv(qs)
                    emit_attn(0, qs)
                    emit_proj(0, qs)
            # ---- phase 2: head-1 attention + final projections ----
            with nc.named_scope("h1"):
                for qs in range(NQS):
                    emit_attn(1, qs)
                    emit_proj(1, qs)

    nc.compile()
    return nc


def make_in_maps(x: np.ndarray, W_qkv: np.ndarray, W_out: np.ndarray):
    """Host-side shard prep: per-core input dict."""
    x = np.asarray(x, dtype=np.float32)
    W_qkv = np.asarray(W_qkv, dtype=np.float32)
    W_out = np.asarray(W_out, dtype=np.float32)

    # causal mask for diagonal blocks: mask[jp, 512*g + ql] = 1.0 if 128*g + jp <= ql
    jp = np.arange(128)[:, None]
    ql = np.arange(512)[None, :]
    mask = np.zeros((128, 2048), np.float32)
    for g in range(4):
        mask[:, 512 * g : 512 * (g + 1)] = (128 * g + jp <= ql).astype(np.float32)
    mask = _to_dt(mask, DT_O)

    # vx init pattern: per 128-wide j-block, col 32 = 1.0, rest 0
    vinit = np.zeros((128, T), np.float32)
    vinit[:, 32::128] = 1.0
    vinit = _to_dt(vinit, DT_O)

    in_maps = []
    for c in range(NCORES):
        b = c // 2
        h0 = 2 * (c % 2)
        xt = _to_dt(x[b].T, DT_S)
        m = {"xt": xt, "mask": mask, "vinit": vinit}
        for i, h in enumerate((h0, h0 + 1)):
            wq_pad = np.zeros((D, 128), np.float32)
            wq_pad[:, 0:32] = W_qkv[32 * h : 32 * (h + 1), :].T
            wk_pad = np.zeros((D, 128), np.float32)
            wk_pad[:, 0:32] = W_qkv[128 + 32 * h : 128 + 32 * (h + 1), :].T
            wo_pad = np.zeros((128, 128), np.float32)
            wo_pad[0:32, :] = W_out[:, 32 * h : 32 * (h + 1)].T
            m[f"wq{i}"] = _to_dt(wq_pad, DT_S)
            m[f"wk{i}"] = _to_dt(wk_pad, DT_S)
            m[f"wo{i}"] = _to_dt(wo_pad, DT_S)
        m["wv"] = _to_dt(W_qkv[256 + 32 * h0 : 256 + 32 * h0 + 64, :].T, DT_S)
        in_maps.append(m)
    return in_maps


_PROGRAM_CACHE = {}


def kernel(x: np.ndarray, W_qkv: np.ndarray, W_out: np.ndarray, _trace=False, _tmpdir=None) -> np.ndarray:
    if "nc" not in _PROGRAM_CACHE:
        _PROGRAM_CACHE["nc"] = build_program()
    nc = _PROGRAM_CACHE["nc"]

    in_maps = make_in_maps(x, W_qkv, W_out)
    res = bass_utils.run_bass_kernel_spmd(
        nc, in_maps, core_ids=list(range(NCORES)), trace=_trace, tmpdir=_tmpdir
    )
    out = np.empty((B, T, D), np.float32)
    for b in range(B):
        out[b] = res.results[2 * b]["y"] + res.results[2 * b + 1]["y"]
    if _trace:
        kernel.last_result = res
    return out


# revision 10
# speedup vs baseline: 1.0521x; 1.0521x over previous
"""Trainium2 Bass kernel: causal multi-head self-attention (B=4, T=4096, D=128, H=4, dh=32).

Sharding: 8 cores = 4 batches x 2 head-pairs. Core c handles batch c//2, heads
{2*(c%2), 2*(c%2)+1}. Each core returns a partial output projection (its 2 heads'
contribution); the host sums the two partials per batch.

Algorithm per (head, q-super of 512 queries):
  scores^T[j, q] = K^T-block (zero-padded K=128 lhsT) @ Q^T  -> PSUM
  E = exp(scores^T * 1/sqrt(dh))  (fp32r out, no max-subtraction: scores are O(3))
  E *= causal mask on diagonal blocks
  O^T[d|l, q] += [V | ones | 0]^T-block @ E   (accumulated over j-blocks in PSUM;
                                               row 32 = softmax denominator l)
  Y_h[q, o] = (O^T slice)^T @ W_out-slice ; out = Y_h0/l0 + Y_h1/l1 (per-partition scale)
"""

import math
import numpy as np

import concourse.bass as bass
import concourse.bacc as bacc
import concourse.mybir as mybir
import concourse.tile as tile
from concourse import bass_utils

F32 = mybir.dt.float32
F32R = mybir.dt.float32r
Exp = mybir.ActivationFunctionType.Exp

B, T, D = 4, 4096, 128
H, DH = 4, 32
NCORES = 8
NQS = T // 512          # 8 q-supers
NJB = T // 128          # 32 j-blocks
GROUP = 3               # j-blocks per S-psum group (3 banks per slot, double buffered)
SCALE = 1.0 / math.sqrt(DH)

# matmul dtypes (switchable): S-stage (Q/K) and O-stage (V/E)
DT_S = F32R
DT_O = F32R


def _round_f32r(x: np.ndarray) -> np.ndarray:
    """Round-to-nearest-even fp32 -> fp32r (11 explicit mantissa bits, low 12 dropped)."""
    u = np.ascontiguousarray(x, dtype=np.float32).view(np.uint32)
    half = np.uint32(1 << 11)
    lsb = (u >> np.uint32(12)) & np.uint32(1)
    u = ((u + half - np.uint32(1) + lsb) >> np.uint32(12)) << np.uint32(12)
    return u.view(np.float32)


def _to_dt(x: np.ndarray, dt) -> np.ndarray:
    if dt == F32R:
        return _round_f32r(x)
    return np.ascontiguousarray(x, dtype=np.float32)


def build_program() -> bacc.Bacc:
    nc = bacc.Bacc("TRN2", target_bir_lowering=False, debug=False, num_devices=NCORES)

    # ---- DRAM I/O (per core) ----
    xt_d = nc.dram_tensor("xt", [D, T], DT_S, kind="ExternalInput").ap()
    wq_d = [nc.dram_tensor(f"wq{h}", [D, 128], DT_S, kind="ExternalInput").ap() for h in range(2)]
    wk_d = [nc.dram_tensor(f"wk{h}", [D, 128], DT_S, kind="ExternalInput").ap() for h in range(2)]
    wv_d = nc.dram_tensor("wv", [D, 64], DT_S, kind="ExternalInput").ap()
    wo_d = [nc.dram_tensor(f"wo{h}", [128, 128], DT_S, kind="ExternalInput").ap() for h in range(2)]
    mask_d = nc.dram_tensor("mask", [128, 896], DT_O, kind="ExternalInput").ap()
    vinit_d = nc.dram_tensor("vinit", [128, T], DT_O, kind="ExternalInput").ap()
    y_d = nc.dram_tensor("y", [T, D], F32, kind="ExternalOutput").ap()
    l_d = nc.dram_tensor("ldram", [2, NQS, 512], F32, kind="Internal").ap()

    with tile.TileContext(nc) as tc:
        with (
            tc.tile_pool(name="const", bufs=1) as cpool,
            tc.tile_pool(name="epool", bufs=3) as epool,
            tc.tile_pool(name="work", bufs=3) as wpool,
            tc.tile_pool(name="psS", bufs=2, space="PSUM") as psS,
            tc.tile_pool(name="psO", bufs=1, space="PSUM") as psO,
            tc.tile_pool(name="psB", bufs=1, space="PSUM") as psB,
        ):
            # ---- persistent SBUF ----
            xt = cpool.tile([D, T], DT_S)
            wq = [cpool.tile([D, 128], DT_S, name=f"wq{h}", tag=f"wq{h}") for h in range(2)]
            wk = [cpool.tile([D, 128], DT_S, name=f"wk{h}", tag=f"wk{h}") for h in range(2)]
            wv = cpool.tile([D, 64], DT_S)
            wo = [cpool.tile([128, 128], DT_S, name=f"wo{h}", tag=f"wo{h}") for h in range(2)]
            mask = cpool.tile([128, 896], DT_O)
            qt = [cpool.tile([128, T], DT_S, name=f"qt{h}", tag=f"qt{h}") for h in range(2)]
            ktz = [cpool.tile([128, T], DT_S, name=f"ktz{h}", tag=f"ktz{h}") for h in range(2)]
            vx = [cpool.tile([128, T], DT_O, name=f"vx{h}", tag=f"vx{h}") for h in range(2)]
            osb = [cpool.tile([128, T], DT_S, name=f"osb{h}", tag=f"osb{h}") for h in range(2)]
            lcol = [cpool.tile([128, NJB], F32, name=f"lcol{h}", tag=f"lcol{h}") for h in range(2)]
            rl = [cpool.tile([128, NJB], F32, name=f"rl{h}", tag=f"rl{h}") for h in range(2)]
            ytmp = cpool.tile([128, T], F32)

            # ---- weight / mask loads ----
            for h in range(2):
                nc.sync.dma_start(wq[h][:, :], wq_d[h][:, :])
                nc.sync.dma_start(wk[h][:, :], wk_d[h][:, :])
                nc.sync.dma_start(wo[h][:, :], wo_d[h][:, :])
            nc.sync.dma_start(wv[:, :], wv_d[:, :])
            nc.sync.dma_start(mask[:, :], mask_d[:, :])
            # vx pre-pattern from host: [0]*32 | 1.0 | [0]*95 per j-block; V values DMAed on top
            for c in range(NQS):
                csl = slice(512 * c, 512 * (c + 1))
                nc.sync.dma_start(vx[0][:, csl], vinit_d[:, csl])
                nc.sync.dma_start(vx[1][:, csl], vinit_d[:, csl])

            def emit_qkv(qs):
                sl = slice(512 * qs, 512 * (qs + 1))
                nc.sync.dma_start(xt[:, sl], xt_d[:, sl])
                for dst, w in ((qt[0], wq[0]), (qt[1], wq[1]), (ktz[0], wk[0]), (ktz[1], wk[1])):
                    p = psB.tile([128, 512], F32, name="p", tag="psb")
                    nc.tensor.matmul(p[:, :], w[:, :], xt[:, sl], start=True, stop=True)
                    nc.vector.tensor_copy(dst[:, sl], p[:, :])
                for jb in range(4 * qs, 4 * qs + 4):
                    jsl = slice(128 * jb, 128 * (jb + 1))
                    p = psB.tile([128, 512], F32, name="p", tag="psb")
                    nc.tensor.matmul(p[:, 0:64], xt[:, jsl], wv[:, :], start=True, stop=True)
                    nc.vector.tensor_copy(vx[0][:, 128 * jb : 128 * jb + 32], p[:, 0:32])
                    nc.vector.tensor_copy(vx[1][:, 128 * jb : 128 * jb + 32], p[:, 32:64])

            def emit_attn(h, qs):
                qsl = slice(512 * qs, 512 * (qs + 1))
                njb = 4 * (qs + 1)
                o_ps = psO.tile([128, 512], F32, name="o_ps", tag="po")
                jb0 = 0
                while jb0 < njb:
                    n = min(GROUP, njb - jb0)
                    s_ps = psS.tile([128, 512 * GROUP], F32, name="s_ps", tag="s")
                    for k in range(n):
                        jb = jb0 + k
                        nc.tensor.matmul(
                            s_ps[:, 512 * k : 512 * (k + 1)],
                            ktz[h][:, 128 * jb : 128 * (jb + 1)],
                            qt[h][:, qsl],
                            start=True, stop=True,
                        )
                    e = epool.tile([128, 512 * GROUP], DT_O, name="e", tag="e")
                    nc.scalar.activation(e[:, 0 : 512 * n], s_ps[:, 0 : 512 * n], Exp, scale=SCALE)
                    for k in range(n):
                        jb = jb0 + k
                        g = jb - 4 * qs
                        if g >= 0:
                            esl = slice(512 * k, 512 * (k + 1))
                            nc.vector.tensor_mul(e[:, esl], e[:, esl], mask[:, 384 - 128 * g : 896 - 128 * g])
                    for k in range(n):
                        jb = jb0 + k
                        nc.tensor.matmul(
                            o_ps[:, :],
                            vx[h][:, 128 * jb : 128 * (jb + 1)],
                            e[:, 512 * k : 512 * (k + 1)],
                            start=(jb == 0), stop=(jb == njb - 1),
                        )
                    jb0 += n
                # evacuate: full 128 rows (rows 33..127 are zeros; row 32 = l)
                nc.vector.tensor_copy(osb[h][:, qsl], o_ps[:, :])
                # l -> per-partition column layout via DRAM bounce + reciprocal
                d1 = nc.sync.dma_start(l_d[h, qs, :], osb[h][32:33, qsl].bitcast(F32))
                lsrc = l_d[h, qs, :].rearrange("(b c) -> c b", c=128)
                d2 = nc.sync.dma_start(lcol[h][:, 4 * qs : 4 * qs + 4], lsrc)
                tile.add_dep_helper(d2.ins, d1.ins, reason="l dram bounce RAW")
                nc.vector.reciprocal(rl[h][:, 4 * qs : 4 * qs + 4], lcol[h][:, 4 * qs : 4 * qs + 4])

            def emit_proj(h, qs):
                for qb in range(4 * qs, 4 * qs + 4):
                    bsl = slice(128 * qb, 128 * (qb + 1))
                    p = psB.tile([128, 512], F32, name="p", tag="psb")
                    nc.tensor.matmul(p[:, 0:128], osb[h][:, bsl], wo[h][:, :], start=True, stop=True)
                    if h == 0:
                        nc.vector.tensor_scalar_mul(ytmp[:, bsl], p[:, 0:128], rl[0][:, qb : qb + 1])
                    else:
                        ty = wpool.tile([128, 128], F32, name="ty", tag="ty")
                        nc.vector.scalar_tensor_tensor(
                            ty[:, :], p[:, 0:128], rl[1][:, qb : qb + 1], ytmp[:, bsl],
                            op0=mybir.AluOpType.mult, op1=mybir.AluOpType.add,
                        )
                        nc.sync.dma_start(y_d[bsl, :], ty[:, :])

            # ---- phase 1: qkv (interleaved) + head-0 attention + head-0 projections ----
            with nc.named_scope("h0"):
                for qs in range(NQS):
                    emit_qkv(qs)
                    emit_attn(0, qs)
                    emit_proj(0, qs)
            # ---- phase 2: head-1 attention + final projections ----
            with nc.named_scope("h1"):
                for qs in range(NQS):
                    emit_attn(1, qs)
                    emit_proj(1, qs)

    nc.compile()
    return nc


def make_in_maps(x: np.ndarray, W_qkv: np.ndarray, W_out: np.ndarray):
    """Host-side shard prep: per-core input dict."""
    x = np.asarray(x, dtype=np.float32)
    W_qkv = np.asarray(W_qkv, dtype=np.float32)
    W_out = np.asarray(W_out, dtype=np.float32)

    # sliding causal mask master: master[jp, c] = 1.0 if jp <= c - 384
    # diagonal-block g uses master[:, 384-128g : 896-128g] == (128g + jp <= ql)
    jp = np.arange(128)[:, None]
    cc = np.arange(896)[None, :]
    mask = (jp <= cc - 384).astype(np.float32)
    mask = _to_dt(mask, DT_O)

    # vx init pattern: per 128-wide j-block, col 32 = 1.0, rest 0
    vinit = np.zeros((128, T), np.float32)
    vinit[:, 32::128] = 1.0
    vinit = _to_dt(vinit, DT_O)

    in_maps = []
    for c in range(NCORES):
        b = c // 2
        h0 = 2 * (c % 2)
        xt = _to_dt(x[b].T, DT_S)
        m = {"xt": xt, "mask": mask, "vinit": vinit}
        for i, h in enumerate((h0, h0 + 1)):
            wq_pad = np.zeros((D, 128), np.float32)
            wq_pad[:, 0:32] = W_qkv[32 * h : 32 * (h + 1), :].T
            wk_pad = np.zeros((D, 128), np.float32)
            wk_pad[:, 0:32] = W_qkv[128 + 32 * h : 128 + 32 * (h + 1), :].T
            wo_pad = np.zeros((128, 128), np.float32)
            wo_pad[0:32, :] = W_out[:, 32 * h : 32 * (h + 1)].T
            m[f"wq{i}"] = _to_dt(wq_pad, DT_S)
            m[f"wk{i}"] = _to_dt(wk_pad, DT_S)
            m[f"wo{i}"] = _to_dt(wo_pad, DT_S)
        m["wv"] = _to_dt(W_qkv[256 + 32 * h0 : 256 + 32 * h0 + 64, :].T, DT_S)
        in_maps.append(m)
    return in_maps


_PROGRAM_CACHE = {}


def kernel(x: np.ndarray, W_qkv: np.ndarray, W_out: np.ndarray, _trace=False, _tmpdir=None) -> np.ndarray:
    if "nc" not in _PROGRAM_CACHE:
        _PROGRAM_CACHE["nc"] = build_program()
    nc = _PROGRAM_CACHE["nc"]

    in_maps = make_in_maps(x, W_qkv, W_out)
    res = bass_utils.run_bass_kernel_spmd(
        nc, in_maps, core_ids=list(range(NCORES)), trace=_trace, tmpdir=_tmpdir
    )
    out = np.empty((B, T, D), np.float32)
    for b in range(B):
        out[b] = res.results[2 * b]["y"] + res.results[2 * b + 1]["y"]
    if _trace:
        kernel.last_result = res
    return out


# revision 11
# speedup vs baseline: 1.0629x; 1.0102x over previous
"""Trainium2 Bass kernel: causal multi-head self-attention (B=4, T=4096, D=128, H=4, dh=32).

Sharding: 8 cores = 4 batches x 2 head-pairs. Core c handles batch c//2, heads
{2*(c%2), 2*(c%2)+1}. Each core returns a partial output projection (its 2 heads'
contribution); the host sums the two partials per batch.

Algorithm per (head, q-super of 512 queries):
  scores^T[j, q] = K^T-block (zero-padded K=128 lhsT) @ Q^T  -> PSUM
  E = exp(scores^T * 1/sqrt(dh))  (fp32r out, no max-subtraction: scores are O(3))
  E *= causal mask on diagonal blocks
  O^T[d|l, q] += [V | ones | 0]^T-block @ E   (accumulated over j-blocks in PSUM;
                                               row 32 = softmax denominator l)
  Y_h[q, o] = (O^T slice)^T @ W_out-slice ; out = Y_h0/l0 + Y_h1/l1 (per-partition scale)
"""

import math
import numpy as np

import concourse.bass as bass
import concourse.bacc as bacc
import concourse.mybir as mybir
import concourse.tile as tile
from concourse import bass_utils

F32 = mybir.dt.float32
F32R = mybir.dt.float32r
Exp = mybir.ActivationFunctionType.Exp

B, T, D = 4, 4096, 128
H, DH = 4, 32
NCORES = 8
NQS = T // 512          # 8 q-supers
NJB = T // 128          # 32 j-blocks
GROUP = 3               # j-blocks per S-psum group (3 banks per slot, double buffered)
SCALE = 1.0 / math.sqrt(DH)

# matmul dtypes (switchable): S-stage (Q/K) and O-stage (V/E)
DT_S = F32R
DT_O = F32R


def _round_f32r(x: np.ndarray) -> np.ndarray:
    """Round-to-nearest-even fp32 -> fp32r (11 explicit mantissa bits, low 12 dropped)."""
    u = np.ascontiguousarray(x, dtype=np.float32).view(np.uint32)
    half = np.uint32(1 << 11)
    lsb = (u >> np.uint32(12)) & np.uint32(1)
    u = ((u + half - np.uint32(1) + lsb) >> np.uint32(12)) << np.uint32(12)
    return u.view(np.float32)


def _to_dt(x: np.ndarray, dt) -> np.ndarray:
    if dt == F32R:
        return _round_f32r(x)
    return np.ascontiguousarray(x, dtype=np.float32)


def build_program() -> bacc.Bacc:
    nc = bacc.Bacc("TRN2", target_bir_lowering=False, debug=False, num_devices=NCORES)

    # ---- DRAM I/O (per core) ----
    xt_d = nc.dram_tensor("xt", [D, T], DT_S, kind="ExternalInput").ap()
    wq_d = [nc.dram_tensor(f"wq{h}", [D, 128], DT_S, kind="ExternalInput").ap() for h in range(2)]
    wk_d = [nc.dram_tensor(f"wk{h}", [D, 128], DT_S, kind="ExternalInput").ap() for h in range(2)]
    wv_d = nc.dram_tensor("wv", [D, 64], DT_S, kind="ExternalInput").ap()
    wo_d = [nc.dram_tensor(f"wo{h}", [128, 128], DT_S, kind="ExternalInput").ap() for h in range(2)]
    mask_d = nc.dram_tensor("mask", [128, 896], DT_O, kind="ExternalInput").ap()
    vinit_d = nc.dram_tensor("vinit", [128, T], DT_O, kind="ExternalInput").ap()
    y_d = nc.dram_tensor("y", [T, D], F32, kind="ExternalOutput").ap()
    l_d = nc.dram_tensor("ldram", [2, NQS, 512], F32, kind="Internal").ap()

    with tile.TileContext(nc) as tc:
        with (
            tc.tile_pool(name="const", bufs=1) as cpool,
            tc.tile_pool(name="epool", bufs=3) as epool,
            tc.tile_pool(name="work", bufs=3) as wpool,
            tc.tile_pool(name="psS", bufs=2, space="PSUM") as psS,
            tc.tile_pool(name="psO", bufs=1, space="PSUM") as psO,
            tc.tile_pool(name="psB", bufs=1, space="PSUM") as psB,
        ):
            # ---- persistent SBUF ----
            xt = cpool.tile([D, T], DT_S)
            wq = [cpool.tile([D, 128], DT_S, name=f"wq{h}", tag=f"wq{h}") for h in range(2)]
            wk = [cpool.tile([D, 128], DT_S, name=f"wk{h}", tag=f"wk{h}") for h in range(2)]
            wv = cpool.tile([D, 64], DT_S)
            wo = [cpool.tile([128, 128], DT_S, name=f"wo{h}", tag=f"wo{h}") for h in range(2)]
            mask = cpool.tile([128, 896], DT_O)
            qt = [cpool.tile([128, T], DT_S, name=f"qt{h}", tag=f"qt{h}") for h in range(2)]
            ktz = [cpool.tile([128, T], DT_S, name=f"ktz{h}", tag=f"ktz{h}") for h in range(2)]
            vx = [cpool.tile([128, T], DT_O, name=f"vx{h}", tag=f"vx{h}") for h in range(2)]
            osb = [cpool.tile([128, T], DT_S, name=f"osb{h}", tag=f"osb{h}") for h in range(2)]
            lcol = [cpool.tile([128, NJB], F32, name=f"lcol{h}", tag=f"lcol{h}") for h in range(2)]
            rl = [cpool.tile([128, NJB], F32, name=f"rl{h}", tag=f"rl{h}") for h in range(2)]
            ytmp = cpool.tile([128, T], F32)

            # ---- weight / mask loads ----
            for h in range(2):
                nc.sync.dma_start(wq[h][:, :], wq_d[h][:, :])
                nc.sync.dma_start(wk[h][:, :], wk_d[h][:, :])
                nc.sync.dma_start(wo[h][:, :], wo_d[h][:, :])
            nc.sync.dma_start(wv[:, :], wv_d[:, :])
            nc.sync.dma_start(mask[:, :], mask_d[:, :])
            # vx pre-pattern from host: [0]*32 | 1.0 | [0]*95 per j-block; V values DMAed
            # on the gpsimd SWDGE queue so they don't block the SP HW queue (xt chunks).
            for c in range(NQS):
                csl = slice(512 * c, 512 * (c + 1))
                nc.gpsimd.dma_start(vx[0][:, csl], vinit_d[:, csl])
                nc.gpsimd.dma_start(vx[1][:, csl], vinit_d[:, csl])

            def emit_qkv(qs):
                sl = slice(512 * qs, 512 * (qs + 1))
                nc.sync.dma_start(xt[:, sl], xt_d[:, sl])
                for dst, w in ((qt[0], wq[0]), (qt[1], wq[1]), (ktz[0], wk[0]), (ktz[1], wk[1])):
                    p = psB.tile([128, 512], F32, name="p", tag="psb")
                    nc.tensor.matmul(p[:, :], w[:, :], xt[:, sl], start=True, stop=True)
                    nc.vector.tensor_copy(dst[:, sl], p[:, :])
                for jb in range(4 * qs, 4 * qs + 4):
                    jsl = slice(128 * jb, 128 * (jb + 1))
                    p = psB.tile([128, 512], F32, name="p", tag="psb")
                    nc.tensor.matmul(p[:, 0:64], xt[:, jsl], wv[:, :], start=True, stop=True)
                    nc.vector.tensor_copy(vx[0][:, 128 * jb : 128 * jb + 32], p[:, 0:32])
                    nc.vector.tensor_copy(vx[1][:, 128 * jb : 128 * jb + 32], p[:, 32:64])

            def emit_attn(h, qs):
                qsl = slice(512 * qs, 512 * (qs + 1))
                njb = 4 * (qs + 1)
                o_ps = psO.tile([128, 512], F32, name="o_ps", tag="po")
                jb0 = 0
                while jb0 < njb:
                    n = min(GROUP, njb - jb0)
                    s_ps = psS.tile([128, 512 * GROUP], F32, name="s_ps", tag="s")
                    for k in range(n):
                        jb = jb0 + k
                        nc.tensor.matmul(
                            s_ps[:, 512 * k : 512 * (k + 1)],
                            ktz[h][:, 128 * jb : 128 * (jb + 1)],
                            qt[h][:, qsl],
                            start=True, stop=True,
                        )
                    e = epool.tile([128, 512 * GROUP], DT_O, name="e", tag="e")
                    nc.scalar.activation(e[:, 0 : 512 * n], s_ps[:, 0 : 512 * n], Exp, scale=SCALE)
                    for k in range(n):
                        jb = jb0 + k
                        g = jb - 4 * qs
                        if g >= 0:
                            esl = slice(512 * k, 512 * (k + 1))
                            nc.vector.tensor_mul(e[:, esl], e[:, esl], mask[:, 384 - 128 * g : 896 - 128 * g])
                    for k in range(n):
                        jb = jb0 + k
                        nc.tensor.matmul(
                            o_ps[:, :],
                            vx[h][:, 128 * jb : 128 * (jb + 1)],
                            e[:, 512 * k : 512 * (k + 1)],
                            start=(jb == 0), stop=(jb == njb - 1),
                        )
                    jb0 += n
                # evacuate: full 128 rows (rows 33..127 are zeros; row 32 = l)
                nc.vector.tensor_copy(osb[h][:, qsl], o_ps[:, :])
                # l -> per-partition column layout via DRAM bounce + reciprocal
                d1 = nc.sync.dma_start(l_d[h, qs, :], osb[h][32:33, qsl].bitcast(F32))
                lsrc = l_d[h, qs, :].rearrange("(b c) -> c b", c=128)
                d2 = nc.sync.dma_start(lcol[h][:, 4 * qs : 4 * qs + 4], lsrc)
                tile.add_dep_helper(d2.ins, d1.ins, reason="l dram bounce RAW")
                nc.vector.reciprocal(rl[h][:, 4 * qs : 4 * qs + 4], lcol[h][:, 4 * qs : 4 * qs + 4])

            def emit_proj(h, qs):
                for qb in range(4 * qs, 4 * qs + 4):
                    bsl = slice(128 * qb, 128 * (qb + 1))
                    p = psB.tile([128, 512], F32, name="p", tag="psb")
                    nc.tensor.matmul(p[:, 0:128], osb[h][:, bsl], wo[h][:, :], start=True, stop=True)
                    if h == 0:
                        nc.vector.tensor_scalar_mul(ytmp[:, bsl], p[:, 0:128], rl[0][:, qb : qb + 1])
                    else:
                        ty = wpool.tile([128, 128], F32, name="ty", tag="ty")
                        nc.vector.scalar_tensor_tensor(
                            ty[:, :], p[:, 0:128], rl[1][:, qb : qb + 1], ytmp[:, bsl],
                            op0=mybir.AluOpType.mult, op1=mybir.AluOpType.add,
                        )
                        nc.sync.dma_start(y_d[bsl, :], ty[:, :])

            # ---- phase 1: qkv (interleaved) + head-0 attention + head-0 projections ----
            with nc.named_scope("h0"):
                for qs in range(NQS):
                    emit_qkv(qs)
                    emit_attn(0, qs)
                    emit_proj(0, qs)
            # ---- phase 2: head-1 attention + final projections ----
            with nc.named_scope("h1"):
                for qs in range(NQS):
                    emit_attn(1, qs)
                    emit_proj(1, qs)

    nc.compile()
    return nc


def make_in_maps(x: np.ndarray, W_qkv: np.ndarray, W_out: np.ndarray):
    """Host-side shard prep: per-core input dict."""
    x = np.asarray(x, dtype=np.float32)
    W_qkv = np.asarray(W_qkv, dtype=np.float32)
    W_out = np.asarray(W_out, dtype=np.float32)

    # sliding causal mask master: master[jp, c] = 1.0 if jp <= c - 384
    # diagonal-block g uses master[:, 384-128g : 896-128g] == (128g + jp <= ql)
    jp = np.arange(128)[:, None]
    cc = np.arange(896)[None, :]
    mask = (jp <= cc - 384).astype(np.float32)
    mask = _to_dt(mask, DT_O)

    # vx init pattern: per 128-wide j-block, col 32 = 1.0, rest 0
    vinit = np.zeros((128, T), np.float32)
    vinit[:, 32::128] = 1.0
    vinit = _to_dt(vinit, DT_O)

    in_maps = []
    for c in range(NCORES):
        b = c // 2
        h0 = 2 * (c % 2)
        xt = _to_dt(x[b].T, DT_S)
        m = {"xt": xt, "mask": mask, "vinit": vinit}
        for i, h in enumerate((h0, h0 + 1)):
            wq_pad = np.zeros((D, 128), np.float32)
            wq_pad[:, 0:32] = W_qkv[32 * h : 32 * (h + 1), :].T
            wk_pad = np.zeros((D, 128), np.float32)
            wk_pad[:, 0:32] = W_qkv[128 + 32 * h : 128 + 32 * (h + 1), :].T
            wo_pad = np.zeros((128, 128), np.float32)
            wo_pad[0:32, :] = W_out[:, 32 * h : 32 * (h + 1)].T
            m[f"wq{i}"] = _to_dt(wq_pad, DT_S)
            m[f"wk{i}"] = _to_dt(wk_pad, DT_S)
            m[f"wo{i}"] = _to_dt(wo_pad, DT_S)
        m["wv"] = _to_dt(W_qkv[256 + 32 * h0 : 256 + 32 * h0 + 64, :].T, DT_S)
        in_maps.append(m)
    return in_maps


_PROGRAM_CACHE = {}


def kernel(x: np.ndarray, W_qkv: np.ndarray, W_out: np.ndarray, _trace=False, _tmpdir=None) -> np.ndarray:
    if "nc" not in _PROGRAM_CACHE:
        _PROGRAM_CACHE["nc"] = build_program()
    nc = _PROGRAM_CACHE["nc"]

    in_maps = make_in_maps(x, W_qkv, W_out)
    res = bass_utils.run_bass_kernel_spmd(
        nc, in_maps, core_ids=list(range(NCORES)), trace=_trace, tmpdir=_tmpdir
    )
    out = np.empty((B, T, D), np.float32)
    for b in range(B):
        out[b] = res.results[2 * b]["y"] + res.results[2 * b + 1]["y"]
    if _trace:
        kernel.last_result = res
    return out
